# revision 20
# baseline (speedup 1.0000x reference)
"""DepthwiseSeparableAttention Trainium2 kernel (8-core SPMD), v3.

Sharding: core c -> (batch b = c//4, head-group g = c%4, 4 heads each).

Design (v3):
- q/k projections are pure 3-stream fp8 DoubleRow matmuls: all three depthwise
  taps are folded into three fp8 weight sets (diag(tap_i) W), and the three
  shifted copies of x are streamed from two fp8 parity copies of x. No conv
  compute and no DVE dependency -> the pair-0 scores and the scalar-engine exp
  stream (the critical path, ~140us of irreducible exp) start at ~15-20us.
- v runs as 2 streams: x @ diag(tap1)Wv in fp8-DR plus a 2-tap conv computed
  on the DVE in bf16 and converted to fp8 on the (otherwise idle) gpsimd.
- attn@v in fp8 DoubleRow (contraction 256 per instruction, 2 cols/cycle);
  softmax denominators via a ones column (head blocks padded 65->80 because
  dual-fp8 ldweights requires width % 16 == 0).
- all remaining PE/DVE/DMA work (m1 projections, v projection, attn@v, per-
  chunk normalization, output projection, y writes) is interleaved between
  score matmuls via a fill-task queue so the exp stream never starves.
- output projection in bf16 (fp8 there costs 5% error - measured), y written
  as bf16 partials, summed on host.
"""
import os
import sys
for _p in ('/opt/trn_rl_repo', '/root/.axon_site/_ro/trn_rl_repo'):
    if os.path.isdir(_p):
        sys.path.insert(0, _p)
        break

import numpy as np
import ml_dtypes

import concourse.bass as bass
import concourse.mybir as mybir
import concourse.tile as tile
from concourse.vector_clock import ScopedClock

BF16 = mybir.dt.bfloat16
F32 = mybir.dt.float32
F8 = mybir.dt.float8e4
AF = mybir.ActivationFunctionType
ALU = mybir.AluOpType
DR = mybir.MatmulPerfMode.DoubleRow

S = 2048          # sequence length
D = 1024          # model dim
DT = 8            # d-tiles of 128
JL = 256          # local head channels (4 heads x 64)
N_CORES = 8

# ---------------------------------------------------------------------------
# walrus in this env allows only ONE sync wait per instruction; split Tile's
# excess waits onto no-fuse NOPs / extra drains.
MAX_WAITS = 1


def _patched_drain_and_barrier(self, tick_clock, wait_clock):
    drain_inst = self.nc.sync.drain()
    wait_clock.add_sem_waits(drain_inst.ins, ScopedClock({None: tick_clock.global_clock}))
    si = drain_inst.ins.sync_info
    if si is not None and len(si.on_wait) > 1:
        waits = list(si.on_wait)
        drain_inst.ins.sync_info = mybir.SyncInfo(on_wait=[waits[0]], on_update=list(si.on_update))
        for w in waits[1:]:
            d2 = self.nc.sync.drain()
            d2.ins.sync_info = mybir.SyncInfo(on_wait=[w], on_update=[])
    self.nc.all_engine_barrier()
    popped = self.nc._tile_sem_poison_stack.pop()
    assert popped is self._sem_poison
    self.nc.clear_and_free_semaphores(list(self.sems.allocated().values()))
    self.nc.all_engine_barrier()


tile.TileContext._drain_and_barrier = _patched_drain_and_barrier


def split_multi_waits(nc):
    n_split = 0
    for f in nc.m.functions:
        for blk in f.blocks:
            il = blk.instructions
            if not any(i.sync_info and len(i.sync_info.on_wait) > MAX_WAITS for i in il):
                continue
            newlist = []
            for inst in il:
                si = inst.sync_info
                if si is not None and len(si.on_wait) > MAX_WAITS:
                    waits = list(si.on_wait)
                    head, tail = waits[:-MAX_WAITS], waits[-MAX_WAITS:]
                    for j, w in enumerate(head):
                        nop = mybir.InstNoOp(
                            name=f"{inst.name}-w{j}",
                            sync_info=mybir.SyncInfo(on_wait=[w], on_update=[]),
                            bass_nofuse=True,
                            engine=inst.engine,
                        )
                        newlist.append(nop)
                        n_split += 1
                    inst.sync_info = mybir.SyncInfo(on_wait=tail, on_update=list(si.on_update))
                newlist.append(inst)
            blk.instructions = newlist
    return n_split


# ---------------------------------------------------------------------------
def build_program(split=True):
    nc = bass.Bass()
    P = {}
    # fp8 x copies, chunk-major [c_in_tile, q_chunk, d_tile, 528]: chunk c
    # holds x[512c-4 .. 512c+524); x8 has x[512c+j] at col 4+j, x8o at col
    # 5+j. 528-byte pitch keeps dual-fp8 slot strides multiples of 16; the
    # bootstrap only needs chunk 0 before its first matmul.
    P['x8'] = nc.declare_dram_parameter("x8", [128, 4, DT, 528], F8, isOutput=False)
    P['x8o'] = nc.declare_dram_parameter("x8o", [128, 4, DT, 528], F8, isOutput=False)
    # fp8 weights, one batched tensor per t: [c_in_tile, tap, d_pair, d_slot, j]
    for t in ("q", "k", "v"):
        P['w8' + t] = nc.declare_dram_parameter(
            "w8" + t, [128, 3, 4, 2, JL], F8, isOutput=False)
    P['pbqk'] = nc.declare_dram_parameter("pbqk", [128, 2, 2], F32, isOutput=False)
    P['bv2'] = nc.declare_dram_parameter("bv2", [1, JL], BF16, isOutput=False)
    P['wo'] = nc.declare_dram_parameter("wo", [128, 2, D], BF16, isOutput=False)
    P['y'] = nc.declare_dram_parameter("y", [D, S], BF16, isOutput=True)
    denom_dram = nc.dram_tensor("denom_scratch", [16, 512], F32)

    with tile.TileContext(nc) as tc:
        import contextlib
        with contextlib.ExitStack() as ctx:
            consts = ctx.enter_context(tc.tile_pool(name="consts", bufs=1))
            persist = ctx.enter_context(tc.tile_pool(name="persist", bufs=1))

            # ---- constants + x: few large DMAs, bootstrap-ordered --------
            # (the sync queue serializes dma issues at ~0.6us each, so batch)
            wt = {}
            x8 = persist.tile([128, 4, DT, 528], F8, name="x8")
            x8o = persist.tile([128, 4, DT, 528], F8, name="x8o")
            wt["k"] = consts.tile([128, 3, 4, 2, JL], F8, name="w8k")
            nc.sync.dma_start(out=wt["k"][:], in_=P['w8k'][:])
            nc.sync.dma_start(out=x8[:, 0, :, :], in_=P['x8'][:, 0, :, :])
            nc.sync.dma_start(out=x8o[:, 0, :, :], in_=P['x8o'][:, 0, :, :])
            pbqk_sb = consts.tile([128, 2, 2], F32, name="pbqk")
            nc.sync.dma_start(out=pbqk_sb[:], in_=P['pbqk'][:])
            pb_sb = {"q": pbqk_sb[:, 0, :], "k": pbqk_sb[:, 1, :]}
            wt["q"] = consts.tile([128, 3, 4, 2, JL], F8, name="w8q")
            nc.sync.dma_start(out=wt["q"][:], in_=P['w8q'][:])
            nc.sync.dma_start(out=x8[:, 1:4, :, :], in_=P['x8'][:, 1:4, :, :])
            nc.sync.dma_start(out=x8o[:, 1:4, :, :], in_=P['x8o'][:, 1:4, :, :])
            wt["v"] = consts.tile([128, 3, 4, 2, JL], F8, name="w8v")
            nc.sync.dma_start(out=wt["v"][:], in_=P['w8v'][:])
            bv2_sb = consts.tile([1, JL], BF16)
            nc.sync.dma_start(out=bv2_sb[:], in_=P['bv2'][:])
            wo_sb = consts.tile([128, 2, D], BF16)
            nc.sync.dma_start(out=wo_sb[:], in_=P['wo'][:])
            ones_sb = consts.tile([1, 128], BF16)
            nc.vector.memset(ones_sb[:], 1.0)


            # ---- persistent activations -----------------------------------
            qT = persist.tile([128, 2, S], BF16, name="qT")
            kT = persist.tile([128, 2, S], BF16, name="kT")
            attn_out = persist.tile([128, 8, 512], BF16, name="attn_out")
            # fp8 v tile: [s_in_tile, ks_pair, slot, head*80]; per-head block
            # is [64 v | ones | 15 zero pad] (dual-fp8 width % 16 == 0).
            vx = persist.tile([128, 8, 2, 4 * 80], F8, name="vx")

            p2pool = ctx.enter_context(tc.tile_pool(name="p2pool", bufs=12))
            nrmp = ctx.enter_context(tc.tile_pool(name="nrm", bufs=2))
            ytp = ctx.enter_context(tc.tile_pool(name="ytp", bufs=4))

            # ---- q/k projection: 3-stream fp8 DoubleRow -------------------
            def qkproj_part(pool, psname, t, dst, m, c, dps, ref):
                if dps[0] == 0:
                    ref['ps'] = pool.tile([128, 512], F32, name=psname)
                ps = ref['ps']
                js = slice(128 * m, 128 * (m + 1))
                for dp in dps:
                    sl = slice(2 * dp, 2 * dp + 2)
                    nc.tensor.matmul(
                        ps[:], wt[t][:, 1, dp, :, js],
                        x8[:, c, sl, 4: 516],
                        start=(dp == 0), stop=False, perf_mode=DR)
                    nc.tensor.matmul(
                        ps[:], wt[t][:, 0, dp, :, js],
                        x8o[:, c, sl, 4: 516],
                        start=False, stop=False, perf_mode=DR)
                    nc.tensor.matmul(
                        ps[:], wt[t][:, 2, dp, :, js],
                        x8o[:, c, sl, 6: 518],
                        start=False, stop=(dp == 3), perf_mode=DR)
                if dps[-1] == 3:
                    nc.vector.tensor_scalar(
                        out=dst[:, m, 512 * c: 512 * (c + 1)], in0=ps[:],
                        scalar1=pb_sb[t][:, m: m + 1], scalar2=None, op0=ALU.add)

            def qkproj_chunk(pool, psname, t, dst, m, c):
                qkproj_part(pool, psname, t, dst, m, c, (0, 1, 2, 3), {})

            # ---- bootstrap: chunk 0 of k and q pair-0 ---------------------
            with tc.tile_pool(name="ppqk", bufs=2, space=bass.MemorySpace.PSUM) as ppqk:
                qkproj_chunk(ppqk, "psqk", "k", kT, 0, 0)
                qkproj_chunk(ppqk, "psqk", "q", qT, 0, 0)

            # vx init after the bootstrap so its DVE memsets don't delay the
            # bootstrap's qT/kT drains in the in-order DVE queue
            nc.vector.memset(vx[:], 0.0)
            for h in range(4):
                nc.vector.memset(vx[:, :, :, 80 * h + 64: 80 * h + 65], 1.0)

            # ---- attention + interleaved tail work ------------------------
            fill_tasks = []

            def emit_fill(n=1):
                for _ in range(n):
                    if fill_tasks:
                        task = fill_tasks.pop(0)
                        if task is not None:
                            task()

            with tc.tile_pool(name="ppsc", bufs=2, space=bass.MemorySpace.PSUM) as ppsc, \
                 tc.tile_pool(name="pchunk", bufs=2, space=bass.MemorySpace.PSUM) as pchunk, \
                 tc.tile_pool(name="ppv", bufs=1, space=bass.MemorySpace.PSUM) as ppv, \
                 tc.tile_pool(name="ppo", bufs=1, space=bass.MemorySpace.PSUM) as ppo:

                    # remaining projections share the pchunk PSUM buffers.
                    # k/q pair-0 chunks 1-3 first (whole pieces: the scores
                    # stream needs them within a few ks slots); pair-1 and v
                    # in half-pieces to keep fill granularity ~1us.
                    for c in range(1, 4):
                        fill_tasks.append(
                            (lambda c=c: qkproj_chunk(pchunk, "acc", "k", kT, 0, c)))
                    for c in range(1, 4):
                        fill_tasks.append(
                            (lambda c=c: qkproj_chunk(pchunk, "acc", "q", qT, 0, c)))
                    for t in ("k", "q"):
                        for c in range(4):
                            ref = {}
                            for dps in ((0, 1), (2, 3)):
                                fill_tasks.append(
                                    (lambda t=t, c=c, dps=dps, ref=ref:
                                     qkproj_part(pchunk, "acc", t,
                                                 kT if t == "k" else qT,
                                                 1, c, dps, ref)))

                    # v projection: 3-stream fp8 DoubleRow (streams as lhsT)
                    def vproj_part(st, dps, ref):
                        if dps[0] == 0:
                            ref['ps'] = ppv.tile([128, 256], F32, name="psv")
                        psv = ref['ps']
                        cc, scol = st // 4, 128 * (st % 4)
                        for dp in dps:
                            sl = slice(2 * dp, 2 * dp + 2)
                            nc.tensor.matmul(
                                psv[:], x8[:, cc, sl, 4 + scol: 4 + scol + 128],
                                wt["v"][:, 1, dp, :, :],
                                start=(dp == 0), stop=False, perf_mode=DR)
                            nc.tensor.matmul(
                                psv[:], x8o[:, cc, sl, 4 + scol: 4 + scol + 128],
                                wt["v"][:, 0, dp, :, :],
                                start=False, stop=False, perf_mode=DR)
                            nc.tensor.matmul(
                                psv[:], x8o[:, cc, sl, 6 + scol: 6 + scol + 128],
                                wt["v"][:, 2, dp, :, :],
                                start=False, stop=False, perf_mode=DR)
                        if dps[-1] == 3:
                            nc.tensor.matmul(
                                psv[:], ones_sb[0:1, :], bv2_sb[0:1, :],
                                start=False, stop=True)
                            dst = vx[:, st // 2, st % 2, :].rearrange(
                                "p (h c) -> p h c", h=4)[:, :, 0:64]
                            nc.vector.tensor_copy(
                                dst, psv[:].rearrange("p (h c) -> p h c", h=4))

                    for st in range(16):
                        ref = {}
                        for dps in ((0, 1), (2, 3)):
                            fill_tasks.append(
                                lambda st=st, dps=dps, ref=ref:
                                vproj_part(st, dps, ref))

                    def attn_piece(pair, chunk, kp, acc, p2t):
                        if kp == 0:
                            for hh in range(2):
                                acc[hh] = pchunk.tile([128, 512], F32, name="acc")
                        for hh in range(2):
                            hl = 2 * pair + hh
                            nc.tensor.matmul(
                                acc[hh][0:80, :],
                                vx[:, kp, :, 80 * hl: 80 * hl + 80],
                                p2t[kp][:, :, 512 * hh: 512 * (hh + 1)],
                                start=(kp == 0), stop=(kp == 7),
                                perf_mode=DR)

                    def norm_piece(pair, chunk, acc):
                        idx = 4 * pair + chunk
                        for hh in range(2):
                            den = nrmp.tile([1, 512], F32, name="den", bufs=2)
                            nc.vector.tensor_copy(den[:], acc[hh][64:65, :])
                            nc.sync.dma_start(
                                out=denom_dram[2 * idx + hh: 2 * idx + hh + 1, :],
                                in_=den[:])
                            nc.vector.tensor_copy(
                                attn_out[64 * hh: 64 * (hh + 1), idx, :],
                                acc[hh][0:64, :])
                        bc = nrmp.tile([128, 512], F32, name="bc", bufs=2)
                        for hh in range(2):
                            rr = denom_dram[2 * idx + hh: 2 * idx + hh + 1, :]
                            bc_ap = bass.AP(
                                tensor=rr.tensor, offset=rr.offset,
                                ap=[[0, 64]] + list(rr.ap[1:]))
                            nc.gpsimd.dma_start(
                                out=bc[64 * hh: 64 * (hh + 1), :], in_=bc_ap)
                        rc = nrmp.tile([128, 512], F32, name="rc", bufs=2)
                        nc.vector.reciprocal(rc[:], bc[:])
                        for hh in range(2):
                            nc.vector.tensor_tensor(
                                out=attn_out[64 * hh: 64 * (hh + 1), idx, :],
                                in0=attn_out[64 * hh: 64 * (hh + 1), idx, :],
                                in1=rc[64 * hh: 64 * (hh + 1), :],
                                op=ALU.mult)

                    def outproj_piece(chunk, m, pool=None, pname="pso"):
                        pso = (pool or ppo).tile([128, 512], F32, name=pname)
                        for pr in range(2):
                            nc.tensor.matmul(
                                pso[:],
                                wo_sb[:, pr, 128 * m: 128 * (m + 1)],
                                attn_out[:, 4 * pr + chunk, :],
                                start=(pr == 0), stop=(pr == 1))
                        yt = ytp.tile([128, 512], BF16, name="yt")
                        nc.vector.tensor_copy(yt[:], pso[:])
                        nc.sync.dma_start(
                            out=P['y'][128 * m: 128 * (m + 1),
                                       512 * chunk: 512 * (chunk + 1)],
                            in_=yt[:])

                    # ---- main scores/exp loop -----------------------------
                    for pair in range(2):
                        for chunk in range(4):
                            q0 = 512 * chunk
                            p2t = {}
                            acc = {}
                            last = (pair == 1 and chunk == 3)
                            for ks in range(16):
                                sc = ppsc.tile([128, 1024], F32, name="sc")
                                for hh in range(2):
                                    r0 = 64 * hh
                                    nc.tensor.matmul(
                                        sc[:, 512 * hh: 512 * (hh + 1)],
                                        kT[r0:r0 + 64, pair, 128 * ks: 128 * (ks + 1)],
                                        qT[r0:r0 + 64, pair, q0: q0 + 512],
                                        start=True, stop=True,
                                        tile_position=(r0, 0))
                                if ks % 2 == 0:
                                    p2t[ks // 2] = p2pool.tile(
                                        [128, 2, 1024], F8, name="p2")
                                nc.scalar.activation(
                                    p2t[ks // 2][:, ks % 2, :], sc[:],
                                    AF.Exp, scale=0.125)
                                # in the final chunk, run its own attention
                                # pieces as soon as their exps are ~done so the
                                # post-exp tail stays short
                                if last and ks >= 4 and ks % 2 == 0:
                                    fill_tasks.insert(
                                        0, (lambda kp=ks // 2 - 2:
                                            attn_piece(1, 3, kp, acc, p2t)))
                                emit_fill(1)
                                # drain backlog faster once projections are done
                                if pair == 1 and len(fill_tasks) > 10:
                                    emit_fill(1)
                            for kp in ((6, 7) if last else range(8)):
                                fill_tasks.append(
                                    lambda pair=pair, chunk=chunk, kp=kp, acc=acc, p2t=p2t:
                                    attn_piece(pair, chunk, kp, acc, p2t))
                            fill_tasks.append(
                                lambda pair=pair, chunk=chunk, acc=acc:
                                norm_piece(pair, chunk, acc))
                            # output projection for the PREVIOUS pair-1 chunk:
                            # deferred one chunk so it never waits on the just-
                            # emitted normalization chain.
                            if pair == 1 and chunk >= 1:
                                for m in range(8):
                                    fill_tasks.append(
                                        lambda chunk=chunk - 1, m=m:
                                        outproj_piece(chunk, m))

                    # drain remaining tail work, then the last chunk's output
                    # projection on two alternating PSUM pools (the attention
                    # accumulators are free by now) to pipeline the tail.
                    while fill_tasks:
                        task = fill_tasks.pop(0)
                        if task is not None:
                            task()
                    for m in range(8):
                        if m % 2 == 0:
                            outproj_piece(3, m)
                        else:
                            outproj_piece(3, m, pool=pchunk, pname="acc")

    if split:
        split_multi_waits(nc)
    return nc


# ---------------------------------------------------------------------------
def make_in_maps(x, dwq_w, dwq_b, dwk_w, dwk_b, dwv_w, dwv_b,
                 wq, bq, wk, bk, wv, bv, wo, bo):
    bf = ml_dtypes.bfloat16
    f8 = ml_dtypes.float8_e4m3
    in_maps = []
    xp_cache = {}

    def arr(a, dt=bf):
        return np.ascontiguousarray(a).astype(dt)

    def dpmajor(wT, dt=f8):
        # [D, JL] -> [128, 4, 2, JL]  (c_in_tile, d_pair, d_slot, j)
        return arr(wT.reshape(4, 2, 128, JL).transpose(2, 0, 1, 3), dt)

    for c in range(N_CORES):
        b, g = divmod(c, 4)
        js = slice(JL * g, JL * (g + 1))
        if b not in xp_cache:
            xT = x[b].T.astype(np.float32)          # [D, S]
            xpad = np.zeros((D, S + 8), np.float32)
            xpad[:, 4:S + 4] = xT                   # x[s] at col 4+s
            x8 = np.zeros((D, 4, 528), np.float32)
            x8o = np.zeros((D, 4, 528), np.float32)
            for c in range(4):
                win = xpad[:, 512 * c: 512 * c + 528]
                x8[:, c, :win.shape[1]] = win       # x[512c+j] at col 4+j
                x8o[:, c, 1:1 + win.shape[1] - 1] = win[:, :-1]
            xp_cache[b] = (
                arr(x8.reshape(DT, 128, 4, 528).transpose(1, 2, 0, 3), f8),
                arr(x8o.reshape(DT, 128, 4, 528).transpose(1, 2, 0, 3), f8))
        m = {'x8': xp_cache[b][0], 'x8o': xp_cache[b][1]}
        pbs = {}
        for t, w_, dw_w, dw_b, pb_ in (("q", wq, dwq_w, dwq_b, bq),
                                       ("k", wk, dwk_w, dwk_b, bk),
                                       ("v", wv, dwv_w, dwv_b, bv)):
            wT = w_[js, :].T                                   # [D, JL]
            m['w8' + t] = np.stack(
                [dpmajor(wT * dw_w[:, i:i + 1]) for i in (0, 1, 2)], axis=1)
            pb = dw_b @ wT + pb_[js]
            if t in ("q", "k"):
                pbs[t] = pb.reshape(2, 128).T
            else:
                m['bv2'] = arr(pb.reshape(1, JL))
        m['pbqk'] = arr(np.stack([pbs['q'], pbs['k']], axis=1), np.float32)
        m['wo'] = arr(wo[:, js].T.reshape(2, 128, D).transpose(1, 0, 2))
        in_maps.append(m)
    return in_maps


def gather_output(results, bo):
    B = 2
    out = np.zeros((B, S, D), np.float32)
    for c in range(N_CORES):
        b = c // 4
        out[b] += results[c]['y'].astype(np.float32).T
    out += bo
    return out


# ---------------------------------------------------------------------------
_PROGRAM_CACHE = {}


def kernel(x, dwq_w, dwq_b, dwk_w, dwk_b, dwv_w, dwv_b,
           wq, bq, wk, bk, wv, bv, wo, bo):
    """Full-input entry point: shards across 8 NeuronCores internally."""
    from concourse.bass_utils import run_bass_kernel_spmd

    x = np.asarray(x, np.float32)
    args = dict(x=x,
                dwq_w=np.asarray(dwq_w, np.float32), dwq_b=np.asarray(dwq_b, np.float32),
                dwk_w=np.asarray(dwk_w, np.float32), dwk_b=np.asarray(dwk_b, np.float32),
                dwv_w=np.asarray(dwv_w, np.float32), dwv_b=np.asarray(dwv_b, np.float32),
                wq=np.asarray(wq, np.float32), bq=np.asarray(bq, np.float32),
                wk=np.asarray(wk, np.float32), bk=np.asarray(bk, np.float32),
                wv=np.asarray(wv, np.float32), bv=np.asarray(bv, np.float32),
                wo=np.asarray(wo, np.float32), bo=np.asarray(bo, np.float32))
    if 'nc' not in _PROGRAM_CACHE:
        _PROGRAM_CACHE['nc'] = build_program()
    nc = _PROGRAM_CACHE['nc']
    in_maps = make_in_maps(**args)
    res = run_bass_kernel_spmd(nc, in_maps, list(range(N_CORES)))
    return gather_output(res.results, args['bo']).astype(np.float32)
